# revision 16
# baseline (speedup 1.0000x reference)
"""GatedTSNorm Trainium2 kernel.

Math: the gated EMA y[t] = (1-g~[t])*y[t-1] + g~[t]*v[t] is linear with a
gate shared across channels, so channel-weighted sums commute with it:
    mean[b,t] = EMA(sum_c wa_c x[b,c,t])
    var[b,t]  = EMA(sum_c wb_c (x-mean)^2) = EMA(sb - mean*(2*ub - mean))
with ua/ub/sb the wa/wb-weighted reductions of x and x^2 over C
(softmax weights sum to 1). Output:
    out[c,t] = Wo_w[c] * (x[c,t] - mean[t]) * rsqrt(var[t]+eps) + Wo_b[c]
             = Wo_w[c] * (alpha[t]*x[c,t] + (-alpha[t]*mean[t])) + Wo_b[c]

Sharding: batch B=8 -> one batch per NeuronCore, zero communication.

Per-core pipeline over T chunks (c-major layout, C=512 on 4 partition
tiles):
  DMA in -> ScalarE x^2 -> TensorE reductions (one fused PSUM group,
  stationary [wa|wb|0] on x and [0|0|wb] on x^2, float32r moving) ->
  VectorE tensor_tensor_scan for the two EMAs (tiny per-(b,t) rows;
  elementwise prep runs in a (128, TC/128) packed layout via DRAM-bounce
  repacks so it uses all lanes) -> TensorE broadcasts alpha/-delta rows
  across partitions with a ones-stationary matmul -> one VectorE pass
  p = x * bc_alpha -> TensorE identity-matmul injects p into PSUM
  accumulating onto bc(-delta) -> ScalarE evacuates with the per-channel
  affine fused: out = Wo_w * psum + Wo_b -> DMA out.
"""

import numpy as np

MOMENTUM = 0.05
EPS = 1e-06

B, C, T = 8, 512, 8192
NCT = C // 128          # 4 partition tiles of channels
TC = 1024               # time chunk
SUB = 512               # matmul / psum subchunk
NSC = TC // SUB
PJ = TC // 128          # packed free size per chunk

_PROG_CACHE = {}


def _build_program(t_total):
    import concourse.bacc as bacc
    import concourse.bass as bass
    import concourse.tile as tile
    from concourse import mybir

    f32 = mybir.dt.float32
    f32r = mybir.dt.float32r
    AF = mybir.ActivationFunctionType
    OP = mybir.AluOpType

    nch = t_total // TC

    nc = bacc.Bacc(None, target_bir_lowering=False)

    x_d = nc.dram_tensor("x", [C, t_total], f32r, kind="ExternalInput")
    a_d = nc.dram_tensor("a_row", [1, t_total], f32, kind="ExternalInput")
    gt_d = nc.dram_tensor("gt_pack", [128, t_total // 128], f32,
                          kind="ExternalInput")
    w3a_d = nc.dram_tensor("w3a", [128, NCT, 3], f32r, kind="ExternalInput")
    w3b_d = nc.dram_tensor("w3b", [128, NCT, 3], f32r, kind="ExternalInput")
    wsc_d = nc.dram_tensor("wscale", [128, NCT], f32, kind="ExternalInput")
    wbi_d = nc.dram_tensor("wbias", [128, NCT], f32, kind="ExternalInput")
    out_d = nc.dram_tensor("out", [C, t_total], f32, kind="ExternalOutput")

    x_v = x_d.rearrange("(ct p) t -> p ct t", p=128)
    out_v = out_d.rearrange("(ct p) t -> p ct t", p=128)

    from contextlib import ExitStack

    with tile.TileContext(nc) as tc, ExitStack() as ctx, \
            nc.allow_low_precision(reason="float32r == fp32 bits in SBUF; "
                                   "PE rounds on load"):
        const = ctx.enter_context(tc.tile_pool(name="const", bufs=1))
        xp = ctx.enter_context(tc.tile_pool(name="xp", bufs=3))
        xsqp = ctx.enter_context(tc.tile_pool(name="xsqp", bufs=3))
        pp = ctx.enter_context(tc.tile_pool(name="pp", bufs=3))
        osb = ctx.enter_context(tc.tile_pool(name="osb", bufs=3))
        rows = ctx.enter_context(tc.tile_pool(name="rows", bufs=2))
        pk = ctx.enter_context(tc.tile_pool(name="pk", bufs=3))
        drp = ctx.enter_context(tc.tile_pool(name="drp", bufs=3, space="DRAM"))
        ps_red = ctx.enter_context(
            tc.tile_pool(name="ps_red", bufs=1, space="PSUM"))
        ps_bca = ctx.enter_context(
            tc.tile_pool(name="ps_bca", bufs=1, space="PSUM"))
        ps_out = ctx.enter_context(
            tc.tile_pool(name="ps_out", bufs=1, space="PSUM"))

        # ---- constants ----
        ones_t = const.tile([128, 128], f32)
        nc.vector.memset(ones_t, 1.0)
        ident = const.tile([128, 128], f32r)
        # iota value = f - p; keep where ==0 -> identity matrix
        nc.gpsimd.affine_select(
            out=ident, in_=ones_t, pattern=[[1, 128]],
            compare_op=OP.is_equal, fill=0.0, base=0, channel_multiplier=-1,
        )
        ones_col = const.tile([1, 128], f32r)
        nc.vector.tensor_copy(out=ones_col, in_=ones_t[0:1, :])
        eps_t = const.tile([128, 1], f32)
        nc.vector.memset(eps_t, EPS)

        w3a = const.tile([128, NCT, 3], f32r)
        nc.sync.dma_start(out=w3a, in_=w3a_d[:])
        w3b = const.tile([128, NCT, 3], f32r)
        nc.sync.dma_start(out=w3b, in_=w3b_d[:])
        wsc = const.tile([128, NCT], f32)
        nc.sync.dma_start(out=wsc, in_=wsc_d[:])
        wbi = const.tile([128, NCT], f32)
        nc.sync.dma_start(out=wbi, in_=wbi_d[:])
        gt = const.tile([128, t_total // 128], f32)
        nc.sync.dma_start(out=gt, in_=gt_d[:])

        def pack(row_ap, tag):
            """(1, TC) SBUF row -> (128, PJ) packed, via DRAM bounce."""
            d = drp.tile([TC], f32, tag="d_" + tag)
            nc.sync.dma_start(out=d.rearrange("(o t) -> o t", o=1), in_=row_ap)
            p = pk.tile([128, PJ], f32, tag=tag)
            nc.sync.dma_start(out=p, in_=d.rearrange("(p j) -> p j", p=128))
            return p

        def unpack(pk_ap, tag, dt=f32):
            """(128, PJ) packed -> (1, TC) SBUF row, via DRAM bounce."""
            d = drp.tile([TC], dt, tag="d_" + tag)
            nc.sync.dma_start(out=d.rearrange("(p j) -> p j", p=128), in_=pk_ap)
            r = rows.tile([1, TC], dt, tag=tag)
            nc.sync.dma_start(out=r, in_=d.rearrange("(o t) -> o t", o=1))
            return r

        prev_mean = None
        prev_var = None
        for k in range(nch):
            t0 = k * TC

            x_ch = xp.tile([128, NCT, TC], f32r)
            nc.sync.dma_start(out=x_ch, in_=x_v[:, :, t0:t0 + TC])
            a_ch = rows.tile([1, TC], f32, tag="a")
            nc.sync.dma_start(out=a_ch, in_=a_d[:, t0:t0 + TC])

            # ---- reductions: ua/ub/sb rows in one psum tile ----
            red_ps = ps_red.tile([3, TC], f32)
            for s in range(NSC):
                sl = slice(s * SUB, (s + 1) * SUB)
                xsq = xsqp.tile([128, NCT, SUB], f32r)
                nc.scalar.activation(out=xsq, in_=x_ch[:, :, sl],
                                     func=AF.Square)
                for ct in range(NCT):
                    nc.tensor.matmul(
                        red_ps[:, sl],
                        w3a[:, ct, :],
                        x_ch[:, ct, sl],
                        start=(ct == 0), stop=False,
                    )
                for ct in range(NCT):
                    nc.tensor.matmul(
                        red_ps[:, sl],
                        w3b[:, ct, :],
                        xsq[:, ct, :],
                        start=False, stop=(ct == NCT - 1),
                    )
            red_rows = rows.tile([3, TC], f32, tag="red")
            nc.scalar.activation(out=red_rows, in_=red_ps, func=AF.Copy)

            ua_p = pack(red_rows[0:1, :], "ua")
            ub_p = pack(red_rows[1:2, :], "ub")
            sb_p = pack(red_rows[2:3, :], "sb")
            gt_k = gt[:, k * PJ:(k + 1) * PJ]

            # ---- mean scan ----
            b1_p = pk.tile([128, PJ], f32, tag="b1")
            nc.vector.tensor_tensor(out=b1_p, in0=gt_k, in1=ua_p, op=OP.mult)
            b1_row = unpack(b1_p, "b1r")
            mean_row = rows.tile([1, TC], f32, tag="mean")
            nc.vector.tensor_tensor_scan(
                out=mean_row, data0=a_ch, data1=b1_row,
                initial=(0.0 if k == 0 else prev_mean[:, TC - 1:TC]),
                op0=OP.mult, op1=OP.add,
            )
            mean_p = pack(mean_row, "mean")

            # ---- v = sb - mean*(2*ub - mean); var scan ----
            t1_p = pk.tile([128, PJ], f32, tag="t1")
            nc.vector.scalar_tensor_tensor(
                out=t1_p, in0=ub_p, scalar=2.0, in1=mean_p,
                op0=OP.mult, op1=OP.subtract,
            )
            t2_p = pk.tile([128, PJ], f32, tag="t2")
            nc.vector.tensor_tensor(out=t2_p, in0=t1_p, in1=mean_p, op=OP.mult)
            v_p = pk.tile([128, PJ], f32, tag="v")
            nc.vector.tensor_tensor(out=v_p, in0=sb_p, in1=t2_p,
                                    op=OP.subtract)
            b2_p = pk.tile([128, PJ], f32, tag="b2")
            nc.vector.tensor_tensor(out=b2_p, in0=gt_k, in1=v_p, op=OP.mult)
            b2_row = unpack(b2_p, "b2r")
            var_row = rows.tile([1, TC], f32, tag="var")
            nc.vector.tensor_tensor_scan(
                out=var_row, data0=a_ch, data1=b2_row,
                initial=(0.0 if k == 0 else prev_var[:, TC - 1:TC]),
                op0=OP.mult, op1=OP.add,
            )
            var_p = pack(var_row, "var")
            prev_mean, prev_var = mean_row, var_row

            # ---- alpha = 1/sqrt(var+eps); negd = -alpha*mean ----
            sq_p = pk.tile([128, PJ], f32, tag="sq")
            nc.scalar.activation(out=sq_p, in_=var_p, func=AF.Sqrt,
                                 bias=eps_t)
            al_p = pk.tile([128, PJ], f32r, tag="al")
            nc.vector.reciprocal(out=al_p, in_=sq_p)
            nd_p = pk.tile([128, PJ], f32r, tag="nd")
            nc.vector.scalar_tensor_tensor(
                out=nd_p, in0=al_p, scalar=-1.0, in1=mean_p,
                op0=OP.mult, op1=OP.mult,
            )
            al_row = unpack(al_p, "alr", f32r)
            nd_row = unpack(nd_p, "ndr", f32r)

            # ---- broadcast alpha across partitions ----
            bca = ps_bca.tile([128, TC], f32)
            for s in range(NSC):
                sl = slice(s * SUB, (s + 1) * SUB)
                nc.tensor.matmul(
                    bca[:, sl], ones_col,
                    al_row[:, sl], start=True, stop=True,
                )

            # ---- p = x * bc_alpha; out = ident@p + ones@(-d); affine evac --
            for s in range(NSC):
                sl = slice(s * SUB, (s + 1) * SUB)
                p_s = pp.tile([128, NCT, SUB], f32r)
                bca_sl = bca[:, sl]
                bca_bc = bass.AP(
                    tensor=bca_sl.tensor, offset=bca_sl.offset,
                    ap=[bca_sl.ap[0], [0, NCT], bca_sl.ap[1]],
                )
                nc.vector.scalar_tensor_tensor(
                    out=p_s, in0=x_ch[:, :, sl], scalar=1.0,
                    in1=bca_bc, op0=OP.mult, op1=OP.mult,
                )
                out_sb = osb.tile([128, NCT, SUB], f32)
                ops_l = [ps_out.tile([128, SUB], f32, tag=f"o{ct}",
                                     name=f"ops{ct}")
                         for ct in range(NCT)]
                for ct in range(NCT):
                    nc.tensor.matmul(
                        ops_l[ct], ones_col,
                        nd_row[:, sl], start=True, stop=False,
                    )
                for ct in range(NCT):
                    nc.tensor.matmul(
                        ops_l[ct], ident,
                        p_s[:, ct, :], start=False, stop=True,
                    )
                for ct in range(NCT):
                    nc.scalar.activation(
                        out=out_sb[:, ct, :], in_=ops_l[ct], func=AF.Identity,
                        scale=wsc[:, ct:ct + 1], bias=wbi[:, ct:ct + 1],
                    )
                nc.sync.dma_start(out=out_v[:, :, t0 + s * SUB:t0 + (s + 1) * SUB],
                                  in_=out_sb)

    nc.compile()
    return nc


def _get_program(t_total=T):
    if t_total not in _PROG_CACHE:
        _PROG_CACHE[t_total] = _build_program(t_total)
    return _PROG_CACHE[t_total]


def _host_prep(x, g, Wa_w, Wb_w, Wo_w, Wo_b, t_total):
    """Build per-core input maps (host does only O(B*T + C) work)."""
    x = np.asarray(x, np.float32)
    g = np.asarray(g, np.float32)
    wa = np.asarray(Wa_w, np.float32).reshape(C)
    wb = np.asarray(Wb_w, np.float32).reshape(C)
    wo_w = np.asarray(Wo_w, np.float32).reshape(C)
    wo_b = np.asarray(Wo_b, np.float32).reshape(C)

    def softmax(v):
        e = np.exp(v - v.max())
        return (e / e.sum()).astype(np.float32)

    wa, wb = softmax(wa), softmax(wb)
    zero = np.zeros_like(wa)
    w3a = np.stack([wa, wb, zero], 1).reshape(NCT, 128, 3).transpose(1, 0, 2)
    w3b = np.stack([zero, zero, wb], 1).reshape(NCT, 128, 3).transpose(1, 0, 2)
    wsc = wo_w.reshape(NCT, 128).T
    wbi = wo_b.reshape(NCT, 128).T

    shared = {
        "w3a": np.ascontiguousarray(w3a),
        "w3b": np.ascontiguousarray(w3b),
        "wscale": np.ascontiguousarray(wsc),
        "wbias": np.ascontiguousarray(wbi),
    }
    nch = t_total // TC
    in_maps = []
    for b in range(x.shape[0]):
        gt = (g[b, 0, :] * MOMENTUM).astype(np.float32)
        a_row = (1.0 - gt).reshape(1, t_total)
        gt_pack = np.ascontiguousarray(
            gt.reshape(nch, 128, PJ).transpose(1, 0, 2).reshape(128, -1))
        in_maps.append({
            "x": np.ascontiguousarray(x[b]),
            "a_row": np.ascontiguousarray(a_row),
            "gt_pack": gt_pack,
            **shared,
        })
    return in_maps


LAST_RESULTS = None


def kernel(x, g, Wa_w, Wb_w, Wo_w, Wo_b):
    global LAST_RESULTS
    from concourse.bass_utils import run_bass_kernel_spmd

    t_total = x.shape[2]
    nc = _get_program(t_total)
    in_maps = _host_prep(x, g, Wa_w, Wb_w, Wo_w, Wo_b, t_total)
    n = len(in_maps)
    res = run_bass_kernel_spmd(nc, in_maps, list(range(n)))
    LAST_RESULTS = res
    return np.stack([res.results[i]["out"] for i in range(n)], 0)


# revision 34
# speedup vs baseline: 1.3406x; 1.3406x over previous
"""GatedTSNorm Trainium2 kernel.

Math: the gated EMA y[t] = (1-g~[t])*y[t-1] + g~[t]*v[t] is linear with a
gate shared across channels, so channel-weighted sums commute with it:
    mean[b,t] = EMA(sum_c wa_c x[b,c,t])
    var[b,t]  = EMA(sum_c wb_c (x-mean)^2) = EMA(sb - mean*(2*ub - mean))
with ua/ub/sb the wa/wb-weighted reductions of x and x^2 over C
(softmax weights sum to 1). Output:
    out[c,t] = Wo_w[c] * (x[c,t] - mean[t]) * rsqrt(var[t]+eps) + Wo_b[c]
             = Wo_w[c] * (alpha[t]*x[c,t] + (-alpha[t]*mean[t])) + Wo_b[c]

Sharding: batch B=8 -> one batch per NeuronCore, zero communication.

Per-core pipeline over T chunks (c-major layout, C=512 on 4 partition
tiles):
  DMA in -> ScalarE x^2 -> TensorE reductions (one fused PSUM group,
  stationary [wa|wb|0] on x and [0|0|wb] on x^2, float32r moving) ->
  VectorE tensor_tensor_scan for the two EMAs (tiny per-(b,t) rows;
  elementwise prep runs in a (128, TC/128) packed layout via DRAM-bounce
  repacks so it uses all lanes) -> TensorE broadcasts alpha/-delta rows
  across partitions with a ones-stationary matmul -> one VectorE pass
  p = x * bc_alpha -> TensorE identity-matmul injects p into PSUM
  accumulating onto bc(-delta) -> ScalarE evacuates with the per-channel
  affine fused: out = Wo_w * psum + Wo_b -> DMA out.
"""

import numpy as np

MOMENTUM = 0.05
EPS = 1e-06

B, C, T = 8, 512, 8192
NCT = C // 128          # 4 partition tiles of channels
TC = 1024               # time chunk
SUB = 512               # matmul / psum subchunk
NSC = TC // SUB
PJ = TC // 128          # packed free size per chunk

_PROG_CACHE = {}


def _build_program(t_total, reps=1):
    import concourse.bacc as bacc
    import concourse.bass as bass
    import concourse.tile as tile
    from concourse import mybir

    f32 = mybir.dt.float32
    f32r = mybir.dt.float32r
    AF = mybir.ActivationFunctionType
    OP = mybir.AluOpType

    nch = t_total // TC

    nc = bacc.Bacc(None, target_bir_lowering=False)

    x_d = nc.dram_tensor("x", [C, t_total], f32r, kind="ExternalInput")
    ag_d = nc.dram_tensor("ag_row", [2, t_total], f32, kind="ExternalInput")
    gt_d = nc.dram_tensor("gt_pack", [128, t_total // 128], f32,
                          kind="ExternalInput")
    w3a_d = nc.dram_tensor("w3a", [128, NCT, 3], f32r, kind="ExternalInput")
    w3b_d = nc.dram_tensor("w3b", [128, NCT, 3], f32r, kind="ExternalInput")
    wsc_d = nc.dram_tensor("wscale", [128, NCT], f32, kind="ExternalInput")
    wbi_d = nc.dram_tensor("wbias", [128, NCT], f32, kind="ExternalInput")
    out_d = nc.dram_tensor("out", [C, t_total], f32, kind="ExternalOutput")

    x_v = x_d.rearrange("(ct p) t -> p ct t", p=128)
    out_v = out_d.rearrange("(ct p) t -> p ct t", p=128)

    from contextlib import ExitStack

    with tile.TileContext(nc) as tc, ExitStack() as ctx, \
            nc.allow_low_precision(reason="float32r == fp32 bits in SBUF; "
                                   "PE rounds on load"):
        const = ctx.enter_context(tc.tile_pool(name="const", bufs=1))
        xp = ctx.enter_context(tc.tile_pool(name="xp", bufs=4))
        xsqp = ctx.enter_context(tc.tile_pool(name="xsqp", bufs=1))
        pp = ctx.enter_context(tc.tile_pool(name="pp", bufs=2))
        osb = ctx.enter_context(tc.tile_pool(name="osb", bufs=2))
        rows = ctx.enter_context(tc.tile_pool(name="rows", bufs=3))
        rows4 = ctx.enter_context(tc.tile_pool(name="rows4", bufs=3))
        agp = ctx.enter_context(tc.tile_pool(name="agp", bufs=2))
        pk = ctx.enter_context(tc.tile_pool(name="pk", bufs=6))
        drp = ctx.enter_context(tc.tile_pool(name="drp", bufs=3, space="DRAM"))
        ps_red = ctx.enter_context(
            tc.tile_pool(name="ps_red", bufs=1, space="PSUM"))
        ps_bca = ctx.enter_context(
            tc.tile_pool(name="ps_bca", bufs=1, space="PSUM"))
        ps_out = ctx.enter_context(
            tc.tile_pool(name="ps_out", bufs=1, space="PSUM"))

        # ---- constants ----
        ones_t = const.tile([128, 128], f32)
        nc.vector.memset(ones_t, 1.0)
        ident = const.tile([128, 128], f32r)
        # iota value = f - p; keep where ==0 -> identity matrix
        nc.gpsimd.affine_select(
            out=ident, in_=ones_t, pattern=[[1, 128]],
            compare_op=OP.is_equal, fill=0.0, base=0, channel_multiplier=-1,
        )
        ones_col = const.tile([1, 128], f32r)
        nc.vector.tensor_copy(out=ones_col, in_=ones_t[0:1, :])
        eps_t = const.tile([128, 1], f32)
        nc.vector.memset(eps_t, EPS)

        w3a = const.tile([128, NCT, 3], f32r)
        nc.sync.dma_start(out=w3a, in_=w3a_d[:])
        w3b = const.tile([128, NCT, 3], f32r)
        nc.sync.dma_start(out=w3b, in_=w3b_d[:])
        wsc = const.tile([128, NCT], f32)
        nc.sync.dma_start(out=wsc, in_=wsc_d[:])
        wbi = const.tile([128, NCT], f32)
        nc.sync.dma_start(out=wbi, in_=wbi_d[:])
        gt = const.tile([128, t_total // 128], f32)
        nc.sync.dma_start(out=gt, in_=gt_d[:])

        def pack(eng, row_ap, tag):
            """(1, TC) SBUF row -> (128, PJ) packed; direct partition-
            crossing DMA (APs iterate in matching flat order)."""
            p = pk.tile([128, PJ], f32, tag=tag)
            eng.dma_start(out=p, in_=row_ap)
            return p

        def unpack(eng, pk_ap, tag, dt=f32):
            """(128, PJ) packed -> (1, TC) SBUF row; direct DMA."""
            r = rows.tile([1, TC], dt, tag=tag)
            eng.dma_start(out=r, in_=pk_ap)
            return r

        rep_cm = (tc.For_i(0, reps, 1, staggered_reset=True)
                  if reps > 1 else None)
        if rep_cm is not None:
            rep_cm.__enter__()
        prev_mean = None
        prev_var = None
        st = {}

        def stage_a(k):
            """input + reductions + scans for chunk k."""
            nonlocal prev_mean, prev_var
            t0 = k * TC
            c = {}

            c["x_ch"] = x_ch = xp.tile([128, NCT, TC], f32r, name="x_ch")
            nc.sync.dma_start(out=x_ch, in_=x_v[:, :, t0:t0 + TC])
            ag_ch = agp.tile([1, 2, TC], f32, tag="ag", name="ag_ch")
            nc.sync.dma_start(out=ag_ch, in_=ag_d[:, t0:t0 + TC])
            a_ch = ag_ch[:, 0, :]
            gt_row = ag_ch[:, 1, :]

            red_ps = ps_red.tile([3, TC], f32, name="red_ps")
            for sx in range(NSC):
                sl = slice(sx * SUB, (sx + 1) * SUB)
                xsq = xsqp.tile([128, NCT, SUB], f32r, name="xsq")
                nc.scalar.activation(out=xsq, in_=x_ch[:, :, sl],
                                     func=AF.Square)
                for ct in range(NCT):
                    nc.tensor.matmul(
                        red_ps[:, sl], w3a[:, ct, :], x_ch[:, ct, sl],
                        start=(ct == 0), stop=False,
                    )
                for ct in range(NCT):
                    nc.tensor.matmul(
                        red_ps[:, sl], w3b[:, ct, :], xsq[:, ct, :],
                        start=False, stop=(ct == NCT - 1),
                    )
            red_rows = rows.tile([3, TC], f32, tag="red", name="red_rows")
            nc.scalar.activation(out=red_rows, in_=red_ps, func=AF.Copy)

            ub_p = pack(nc.scalar, red_rows[1:2, :], "ub")
            sb_p = pack(nc.scalar, red_rows[2:3, :], "sb")
            gt_k = gt[:, k * PJ:(k + 1) * PJ]

            b1_row = rows.tile([1, TC], f32, tag="b1r", name="b1_row")
            nc.vector.tensor_tensor(out=b1_row, in0=gt_row,
                                    in1=red_rows[0:1, :], op=OP.mult)
            mean_row = rows4.tile([1, TC], f32, tag="mean", name="mean_row")
            nc.vector.tensor_tensor_scan(
                out=mean_row, data0=a_ch, data1=b1_row,
                initial=(0.0 if k == 0 else prev_mean[:, TC - 1:TC]),
                op0=OP.mult, op1=OP.add,
            )
            c["mean_row"] = mean_row
            mean_p = pack(nc.gpsimd, mean_row, "mean")

            t1_p = pk.tile([128, PJ], f32, tag="t1", name="t1_p")
            nc.vector.scalar_tensor_tensor(
                out=t1_p, in0=ub_p, scalar=2.0, in1=mean_p,
                op0=OP.mult, op1=OP.subtract,
            )
            t2_p = pk.tile([128, PJ], f32, tag="t2", name="t2_p")
            nc.vector.tensor_tensor(out=t2_p, in0=t1_p, in1=mean_p,
                                    op=OP.mult)
            v_p = pk.tile([128, PJ], f32, tag="v", name="v_p")
            nc.vector.tensor_tensor(out=v_p, in0=sb_p, in1=t2_p,
                                    op=OP.subtract)
            b2_p = pk.tile([128, PJ], f32, tag="b2", name="b2_p")
            nc.vector.tensor_tensor(out=b2_p, in0=gt_k, in1=v_p, op=OP.mult)
            b2_row = unpack(nc.gpsimd, b2_p, "b2r")
            var_row = rows4.tile([1, TC], f32, tag="var", name="var_row")
            nc.vector.tensor_tensor_scan(
                out=var_row, data0=a_ch, data1=b2_row,
                initial=(0.0 if k == 0 else prev_var[:, TC - 1:TC]),
                op0=OP.mult, op1=OP.add,
            )
            c["var_p"] = pack(nc.gpsimd, var_row, "var")
            prev_mean, prev_var = mean_row, var_row
            return c

        def stage_b(k, c):
            """alpha/delta + broadcast + normalize + store for chunk k."""
            t0 = k * TC
            x_ch = c["x_ch"]

            sq_p = pk.tile([128, PJ], f32, tag="sq", name="sq_p")
            nc.scalar.activation(out=sq_p, in_=c["var_p"], func=AF.Sqrt,
                                 bias=eps_t)
            al_p = pk.tile([128, PJ], f32r, tag="al", name="al_p")
            nc.vector.reciprocal(out=al_p, in_=sq_p)
            al_row = unpack(nc.gpsimd, al_p, "alr", f32r)
            nd_row = rows.tile([1, TC], f32r, tag="ndr", name="nd_row")
            nc.vector.scalar_tensor_tensor(
                out=nd_row, in0=al_row, scalar=-1.0, in1=c["mean_row"],
                op0=OP.mult, op1=OP.mult,
            )

            bca = ps_bca.tile([128, TC], f32, name="bca")
            for sx in range(NSC):
                sl = slice(sx * SUB, (sx + 1) * SUB)
                nc.tensor.matmul(bca[:, sl], ones_col, al_row[:, sl],
                                 start=True, stop=True)

            for sx in range(NSC):
                sl = slice(sx * SUB, (sx + 1) * SUB)
                p_s = pp.tile([128, NCT, SUB], f32r, name="p_s")
                bca_sl = bca[:, sl]
                bca_bc = bass.AP(
                    tensor=bca_sl.tensor, offset=bca_sl.offset,
                    ap=[bca_sl.ap[0], [0, NCT], bca_sl.ap[1]],
                )
                nc.vector.scalar_tensor_tensor(
                    out=p_s, in0=x_ch[:, :, sl], scalar=1.0,
                    in1=bca_bc, op0=OP.mult, op1=OP.mult,
                )
                out_sb = osb.tile([128, NCT, SUB], f32, name="out_sb")
                ops_l = [ps_out.tile([128, SUB], f32, tag=f"o{ct}",
                                     name=f"ops{ct}")
                         for ct in range(NCT)]
                for ct in range(NCT):
                    nc.tensor.matmul(ops_l[ct], ones_col, nd_row[:, sl],
                                     start=True, stop=False)
                for ct in range(NCT):
                    nc.tensor.matmul(ops_l[ct], ident, p_s[:, ct, :],
                                     start=False, stop=True)
                for ct in range(NCT):
                    nc.scalar.activation(
                        out=out_sb[:, ct, :], in_=ops_l[ct],
                        func=AF.Identity,
                        scale=wsc[:, ct:ct + 1], bias=wbi[:, ct:ct + 1],
                    )
                nc.scalar.dma_start(
                    out=out_v[:, :, t0 + sx * SUB:t0 + (sx + 1) * SUB],
                    in_=out_sb)

        pri_marks = []
        for k in range(nch + 1):
            pri_marks.append(tc.cur_priority)
            if k < nch:
                st[k] = stage_a(k)
            if k >= 1:
                # sort stage B one full iteration later than emitted so
                # in-order engine queues never block early chunk k+1 work
                # behind chunk k-1 tail waits
                span = (pri_marks[-1] - pri_marks[-2]) if k >= 2 else 0
                with tc.high_priority(offset=-span):
                    stage_b(k - 1, st.pop(k - 1))
        if rep_cm is not None:
            rep_cm.__exit__(None, None, None)

    nc.compile()
    return nc


def _get_program(t_total=T, reps=1):
    key = (t_total, reps)
    if key not in _PROG_CACHE:
        _PROG_CACHE[key] = _build_program(t_total, reps)
    return _PROG_CACHE[key]


def _host_prep(x, g, Wa_w, Wb_w, Wo_w, Wo_b, t_total):
    """Build per-core input maps (host does only O(B*T + C) work)."""
    x = np.asarray(x, np.float32)
    g = np.asarray(g, np.float32)
    wa = np.asarray(Wa_w, np.float32).reshape(C)
    wb = np.asarray(Wb_w, np.float32).reshape(C)
    wo_w = np.asarray(Wo_w, np.float32).reshape(C)
    wo_b = np.asarray(Wo_b, np.float32).reshape(C)

    def softmax(v):
        e = np.exp(v - v.max())
        return (e / e.sum()).astype(np.float32)

    wa, wb = softmax(wa), softmax(wb)
    zero = np.zeros_like(wa)
    w3a = np.stack([wa, wb, zero], 1).reshape(NCT, 128, 3).transpose(1, 0, 2)
    w3b = np.stack([zero, zero, wb], 1).reshape(NCT, 128, 3).transpose(1, 0, 2)
    wsc = wo_w.reshape(NCT, 128).T
    wbi = wo_b.reshape(NCT, 128).T

    shared = {
        "w3a": np.ascontiguousarray(w3a),
        "w3b": np.ascontiguousarray(w3b),
        "wscale": np.ascontiguousarray(wsc),
        "wbias": np.ascontiguousarray(wbi),
    }
    nch = t_total // TC
    in_maps = []
    for b in range(x.shape[0]):
        gt = (g[b, 0, :] * MOMENTUM).astype(np.float32)
        ag_row = np.stack([1.0 - gt, gt], 0)
        gt_pack = np.ascontiguousarray(
            gt.reshape(nch, 128, PJ).transpose(1, 0, 2).reshape(128, -1))
        in_maps.append({
            "x": np.ascontiguousarray(x[b]),
            "ag_row": np.ascontiguousarray(ag_row),
            "gt_pack": gt_pack,
            **shared,
        })
    return in_maps


LAST_RESULTS = None


def kernel(x, g, Wa_w, Wb_w, Wo_w, Wo_b):
    global LAST_RESULTS
    from concourse.bass_utils import run_bass_kernel_spmd

    t_total = x.shape[2]
    nc = _get_program(t_total)
    in_maps = _host_prep(x, g, Wa_w, Wb_w, Wo_w, Wo_b, t_total)
    n = len(in_maps)
    res = run_bass_kernel_spmd(nc, in_maps, list(range(n)))
    LAST_RESULTS = res
    return np.stack([res.results[i]["out"] for i in range(n)], 0)


# revision 35
# speedup vs baseline: 2.9213x; 2.1790x over previous
"""GatedTSNorm Trainium2 kernel.

Math: the gated EMA y[t] = (1-g~[t])*y[t-1] + g~[t]*v[t] is linear with a
gate shared across channels, so channel-weighted sums commute with it:
    mean[b,t] = EMA(sum_c wa_c x[b,c,t])
    var[b,t]  = EMA(sum_c wb_c (x-mean)^2) = EMA(sb - mean*(2*ub - mean))
with ua/ub/sb the wa/wb-weighted reductions of x and x^2 over C
(softmax weights sum to 1). Output:
    out[c,t] = Wo_w[c] * (x[c,t] - mean[t]) * rsqrt(var[t]+eps) + Wo_b[c]
             = Wo_w[c] * (alpha[t]*x[c,t] + (-alpha[t]*mean[t])) + Wo_b[c]

Sharding: batch B=8 -> one batch per NeuronCore, zero communication.

Per-core pipeline over T chunks (c-major layout, C=512 on 4 partition
tiles):
  DMA in -> ScalarE x^2 -> TensorE reductions (one fused PSUM group,
  stationary [wa|wb|0] on x and [0|0|wb] on x^2, float32r moving) ->
  VectorE tensor_tensor_scan for the two EMAs (tiny per-(b,t) rows;
  elementwise prep runs in a (128, TC/128) packed layout via DRAM-bounce
  repacks so it uses all lanes) -> TensorE broadcasts alpha/-delta rows
  across partitions with a ones-stationary matmul -> one VectorE pass
  p = x * bc_alpha -> TensorE identity-matmul injects p into PSUM
  accumulating onto bc(-delta) -> ScalarE evacuates with the per-channel
  affine fused: out = Wo_w * psum + Wo_b -> DMA out.
"""

import numpy as np

MOMENTUM = 0.05
EPS = 1e-06

B, C, T = 8, 512, 8192
NCT = C // 128          # 4 partition tiles of channels
TC = 1024               # time chunk
SUB = 512               # matmul / psum subchunk
NSC = TC // SUB
PJ = TC // 128          # packed free size per chunk

_PROG_CACHE = {}


def _build_program(t_total, reps=1):
    import concourse.bacc as bacc
    import concourse.bass as bass
    import concourse.tile as tile
    from concourse import mybir

    f32 = mybir.dt.float32
    f32r = mybir.dt.float32r
    AF = mybir.ActivationFunctionType
    OP = mybir.AluOpType

    nch = t_total // TC

    nc = bacc.Bacc(None, target_bir_lowering=False)

    x_d = nc.dram_tensor("x", [C, t_total], f32r, kind="ExternalInput")
    ag_d = nc.dram_tensor("ag_row", [2, t_total], f32, kind="ExternalInput")
    gt_d = nc.dram_tensor("gt_pack", [128, t_total // 128], f32,
                          kind="ExternalInput")
    w3a_d = nc.dram_tensor("w3a", [128, NCT, 3], f32r, kind="ExternalInput")
    w3b_d = nc.dram_tensor("w3b", [128, NCT, 3], f32r, kind="ExternalInput")
    wsc_d = nc.dram_tensor("wscale", [128, NCT], f32, kind="ExternalInput")
    wbi_d = nc.dram_tensor("wbias", [128, NCT], f32, kind="ExternalInput")
    out_d = nc.dram_tensor("out", [C, t_total], f32, kind="ExternalOutput")

    x_v = x_d.rearrange("(ct p) t -> p ct t", p=128)
    out_v = out_d.rearrange("(ct p) t -> p ct t", p=128)

    from contextlib import ExitStack

    with tile.TileContext(nc) as tc, ExitStack() as ctx, \
            nc.allow_low_precision(reason="float32r == fp32 bits in SBUF; "
                                   "PE rounds on load"):
        const = ctx.enter_context(tc.tile_pool(name="const", bufs=1))
        xp = ctx.enter_context(tc.tile_pool(name="xp", bufs=4))
        xsqp = ctx.enter_context(tc.tile_pool(name="xsqp", bufs=1))
        pp = ctx.enter_context(tc.tile_pool(name="pp", bufs=2))
        osb = ctx.enter_context(tc.tile_pool(name="osb", bufs=2))
        rows = ctx.enter_context(tc.tile_pool(name="rows", bufs=3))
        rows4 = ctx.enter_context(tc.tile_pool(name="rows4", bufs=3))
        agp = ctx.enter_context(tc.tile_pool(name="agp", bufs=2))
        pk = ctx.enter_context(tc.tile_pool(name="pk", bufs=6))
        ps_red = ctx.enter_context(
            tc.tile_pool(name="ps_red", bufs=1, space="PSUM"))
        ps_bca = ctx.enter_context(
            tc.tile_pool(name="ps_bca", bufs=1, space="PSUM"))
        ps_out = ctx.enter_context(
            tc.tile_pool(name="ps_out", bufs=1, space="PSUM"))

        # ---- constants ----
        ones_t = const.tile([128, 128], f32)
        nc.vector.memset(ones_t, 1.0)
        ident = const.tile([128, 128], f32r)
        # iota value = f - p; keep where ==0 -> identity matrix
        nc.gpsimd.affine_select(
            out=ident, in_=ones_t, pattern=[[1, 128]],
            compare_op=OP.is_equal, fill=0.0, base=0, channel_multiplier=-1,
        )
        ones_col = const.tile([1, 128], f32r)
        nc.vector.tensor_copy(out=ones_col, in_=ones_t[0:1, :])
        eps_t = const.tile([128, 1], f32)
        nc.vector.memset(eps_t, EPS)

        w3a = const.tile([128, NCT, 3], f32r)
        nc.sync.dma_start(out=w3a, in_=w3a_d[:])
        w3b = const.tile([128, NCT, 3], f32r)
        nc.sync.dma_start(out=w3b, in_=w3b_d[:])
        wsc = const.tile([128, NCT], f32)
        nc.sync.dma_start(out=wsc, in_=wsc_d[:])
        wbi = const.tile([128, NCT], f32)
        nc.sync.dma_start(out=wbi, in_=wbi_d[:])
        gt = const.tile([128, t_total // 128], f32)
        nc.sync.dma_start(out=gt, in_=gt_d[:])

        def pack(eng, row_ap, tag):
            """(1, TC) SBUF row -> (128, PJ) packed; direct partition-
            crossing DMA (APs iterate in matching flat order)."""
            p = pk.tile([128, PJ], f32, tag=tag)
            eng.dma_start(out=p, in_=row_ap)
            return p

        def unpack(eng, pk_ap, tag, dt=f32):
            """(128, PJ) packed -> (1, TC) SBUF row; direct DMA."""
            r = rows.tile([1, TC], dt, tag=tag)
            eng.dma_start(out=r, in_=pk_ap)
            return r

        rep_cm = (tc.For_i(0, reps, 1, staggered_reset=True)
                  if reps > 1 else None)
        if rep_cm is not None:
            rep_cm.__enter__()
        prev_mean = None
        prev_var = None
        st = {}

        def stage_a(k):
            """input + reductions + scans for chunk k."""
            nonlocal prev_mean, prev_var
            t0 = k * TC
            c = {}

            c["x_ch"] = x_ch = xp.tile([128, NCT, TC], f32r, name="x_ch")
            nc.sync.dma_start(out=x_ch, in_=x_v[:, :, t0:t0 + TC])
            ag_ch = agp.tile([1, 2, TC], f32, tag="ag", name="ag_ch")
            nc.sync.dma_start(out=ag_ch, in_=ag_d[:, t0:t0 + TC])
            a_ch = ag_ch[:, 0, :]
            gt_row = ag_ch[:, 1, :]

            red_ps = ps_red.tile([3, TC], f32, name="red_ps")
            for sx in range(NSC):
                sl = slice(sx * SUB, (sx + 1) * SUB)
                xsq = xsqp.tile([128, NCT, SUB], f32r, name="xsq")
                nc.scalar.activation(out=xsq, in_=x_ch[:, :, sl],
                                     func=AF.Square)
                for ct in range(NCT):
                    nc.tensor.matmul(
                        red_ps[:, sl], w3a[:, ct, :], x_ch[:, ct, sl],
                        start=(ct == 0), stop=False,
                    )
                for ct in range(NCT):
                    nc.tensor.matmul(
                        red_ps[:, sl], w3b[:, ct, :], xsq[:, ct, :],
                        start=False, stop=(ct == NCT - 1),
                    )
            red_rows = rows.tile([3, TC], f32, tag="red", name="red_rows")
            nc.scalar.activation(out=red_rows, in_=red_ps, func=AF.Copy)

            ub_p = pack(nc.scalar, red_rows[1:2, :], "ub")
            sb_p = pack(nc.scalar, red_rows[2:3, :], "sb")
            gt_k = gt[:, k * PJ:(k + 1) * PJ]

            b1_row = rows.tile([1, TC], f32, tag="b1r", name="b1_row")
            nc.vector.tensor_tensor(out=b1_row, in0=gt_row,
                                    in1=red_rows[0:1, :], op=OP.mult)
            mean_row = rows4.tile([1, TC], f32, tag="mean", name="mean_row")
            nc.vector.tensor_tensor_scan(
                out=mean_row, data0=a_ch, data1=b1_row,
                initial=(0.0 if k == 0 else prev_mean[:, TC - 1:TC]),
                op0=OP.mult, op1=OP.add,
            )
            c["mean_row"] = mean_row
            mean_p = pack(nc.gpsimd, mean_row, "mean")

            t1_p = pk.tile([128, PJ], f32, tag="t1", name="t1_p")
            nc.vector.scalar_tensor_tensor(
                out=t1_p, in0=ub_p, scalar=2.0, in1=mean_p,
                op0=OP.mult, op1=OP.subtract,
            )
            t2_p = pk.tile([128, PJ], f32, tag="t2", name="t2_p")
            nc.vector.tensor_tensor(out=t2_p, in0=t1_p, in1=mean_p,
                                    op=OP.mult)
            v_p = pk.tile([128, PJ], f32, tag="v", name="v_p")
            nc.vector.tensor_tensor(out=v_p, in0=sb_p, in1=t2_p,
                                    op=OP.subtract)
            b2_p = pk.tile([128, PJ], f32, tag="b2", name="b2_p")
            nc.vector.tensor_tensor(out=b2_p, in0=gt_k, in1=v_p, op=OP.mult)
            b2_row = unpack(nc.gpsimd, b2_p, "b2r")
            var_row = rows4.tile([1, TC], f32, tag="var", name="var_row")
            nc.vector.tensor_tensor_scan(
                out=var_row, data0=a_ch, data1=b2_row,
                initial=(0.0 if k == 0 else prev_var[:, TC - 1:TC]),
                op0=OP.mult, op1=OP.add,
            )
            c["var_p"] = pack(nc.gpsimd, var_row, "var")
            prev_mean, prev_var = mean_row, var_row
            return c

        def stage_b(k, c):
            """alpha/delta + broadcast + normalize + store for chunk k."""
            t0 = k * TC
            x_ch = c["x_ch"]

            sq_p = pk.tile([128, PJ], f32, tag="sq", name="sq_p")
            nc.scalar.activation(out=sq_p, in_=c["var_p"], func=AF.Sqrt,
                                 bias=eps_t)
            al_p = pk.tile([128, PJ], f32r, tag="al", name="al_p")
            nc.vector.reciprocal(out=al_p, in_=sq_p)
            al_row = unpack(nc.gpsimd, al_p, "alr", f32r)
            nd_row = rows.tile([1, TC], f32r, tag="ndr", name="nd_row")
            nc.vector.scalar_tensor_tensor(
                out=nd_row, in0=al_row, scalar=-1.0, in1=c["mean_row"],
                op0=OP.mult, op1=OP.mult,
            )

            bca = ps_bca.tile([128, TC], f32, name="bca")
            for sx in range(NSC):
                sl = slice(sx * SUB, (sx + 1) * SUB)
                nc.tensor.matmul(bca[:, sl], ones_col, al_row[:, sl],
                                 start=True, stop=True)

            for sx in range(NSC):
                sl = slice(sx * SUB, (sx + 1) * SUB)
                p_s = pp.tile([128, NCT, SUB], f32r, name="p_s")
                bca_sl = bca[:, sl]
                bca_bc = bass.AP(
                    tensor=bca_sl.tensor, offset=bca_sl.offset,
                    ap=[bca_sl.ap[0], [0, NCT], bca_sl.ap[1]],
                )
                nc.vector.scalar_tensor_tensor(
                    out=p_s, in0=x_ch[:, :, sl], scalar=1.0,
                    in1=bca_bc, op0=OP.mult, op1=OP.mult,
                )
                out_sb = osb.tile([128, NCT, SUB], f32, name="out_sb")
                ops_l = [ps_out.tile([128, SUB], f32, tag=f"o{ct}",
                                     name=f"ops{ct}")
                         for ct in range(NCT)]
                for ct in range(NCT):
                    nc.tensor.matmul(ops_l[ct], ones_col, nd_row[:, sl],
                                     start=True, stop=False)
                for ct in range(NCT):
                    nc.tensor.matmul(ops_l[ct], ident, p_s[:, ct, :],
                                     start=False, stop=True)
                for ct in range(NCT):
                    nc.scalar.activation(
                        out=out_sb[:, ct, :], in_=ops_l[ct],
                        func=AF.Identity,
                        scale=wsc[:, ct:ct + 1], bias=wbi[:, ct:ct + 1],
                    )
                nc.scalar.dma_start(
                    out=out_v[:, :, t0 + sx * SUB:t0 + (sx + 1) * SUB],
                    in_=out_sb)

        pri_marks = []
        for k in range(nch + 1):
            pri_marks.append(tc.cur_priority)
            if k < nch:
                st[k] = stage_a(k)
            if k >= 1:
                # sort stage B one full iteration later than emitted so
                # in-order engine queues never block early chunk k+1 work
                # behind chunk k-1 tail waits
                span = (pri_marks[-1] - pri_marks[-2]) if k >= 2 else 0
                with tc.high_priority(offset=-span):
                    stage_b(k - 1, st.pop(k - 1))
        if rep_cm is not None:
            rep_cm.__exit__(None, None, None)

    nc.compile()
    return nc


def _get_program(t_total=T, reps=1):
    key = (t_total, reps)
    if key not in _PROG_CACHE:
        _PROG_CACHE[key] = _build_program(t_total, reps)
    return _PROG_CACHE[key]


def _host_prep(x, g, Wa_w, Wb_w, Wo_w, Wo_b, t_total):
    """Build per-core input maps (host does only O(B*T + C) work)."""
    x = np.asarray(x, np.float32)
    g = np.asarray(g, np.float32)
    wa = np.asarray(Wa_w, np.float32).reshape(C)
    wb = np.asarray(Wb_w, np.float32).reshape(C)
    wo_w = np.asarray(Wo_w, np.float32).reshape(C)
    wo_b = np.asarray(Wo_b, np.float32).reshape(C)

    def softmax(v):
        e = np.exp(v - v.max())
        return (e / e.sum()).astype(np.float32)

    wa, wb = softmax(wa), softmax(wb)
    zero = np.zeros_like(wa)
    w3a = np.stack([wa, wb, zero], 1).reshape(NCT, 128, 3).transpose(1, 0, 2)
    w3b = np.stack([zero, zero, wb], 1).reshape(NCT, 128, 3).transpose(1, 0, 2)
    wsc = wo_w.reshape(NCT, 128).T
    wbi = wo_b.reshape(NCT, 128).T

    shared = {
        "w3a": np.ascontiguousarray(w3a),
        "w3b": np.ascontiguousarray(w3b),
        "wscale": np.ascontiguousarray(wsc),
        "wbias": np.ascontiguousarray(wbi),
    }
    nch = t_total // TC
    in_maps = []
    for b in range(x.shape[0]):
        gt = (g[b, 0, :] * MOMENTUM).astype(np.float32)
        ag_row = np.stack([1.0 - gt, gt], 0)
        gt_pack = np.ascontiguousarray(
            gt.reshape(nch, 128, PJ).transpose(1, 0, 2).reshape(128, -1))
        in_maps.append({
            "x": np.ascontiguousarray(x[b]),
            "ag_row": np.ascontiguousarray(ag_row),
            "gt_pack": gt_pack,
            **shared,
        })
    return in_maps


LAST_RESULTS = None


def kernel(x, g, Wa_w, Wb_w, Wo_w, Wo_b):
    global LAST_RESULTS
    from concourse.bass_utils import run_bass_kernel_spmd

    t_total = x.shape[2]
    nc = _get_program(t_total)
    in_maps = _host_prep(x, g, Wa_w, Wb_w, Wo_w, Wo_b, t_total)
    n = len(in_maps)
    res = run_bass_kernel_spmd(nc, in_maps, list(range(n)))
    LAST_RESULTS = res
    return np.stack([res.results[i]["out"] for i in range(n)], 0)


# revision 37
# speedup vs baseline: 3.0578x; 1.0467x over previous
"""GatedTSNorm Trainium2 kernel.

Math: the gated EMA y[t] = (1-g~[t])*y[t-1] + g~[t]*v[t] is linear with a
gate shared across channels, so channel-weighted sums commute with it:
    mean[b,t] = EMA(sum_c wa_c x[b,c,t])
    var[b,t]  = EMA(sum_c wb_c (x-mean)^2) = EMA(sb - mean*(2*ub - mean))
with ua/ub/sb the wa/wb-weighted reductions of x and x^2 over C
(softmax weights sum to 1). Output:
    out[c,t] = Wo_w[c] * (x[c,t] - mean[t]) * rsqrt(var[t]+eps) + Wo_b[c]
             = Wo_w[c] * (alpha[t]*x[c,t] + (-alpha[t]*mean[t])) + Wo_b[c]

Sharding: batch B=8 -> one batch per NeuronCore, zero communication.

Per-core pipeline over T chunks (c-major layout, C=512 on 4 partition
tiles):
  DMA in -> ScalarE x^2 -> TensorE reductions (one fused PSUM group,
  stationary [wa|wb|0] on x and [0|0|wb] on x^2, float32r moving) ->
  VectorE tensor_tensor_scan for the two EMAs (tiny per-(b,t) rows;
  elementwise prep runs in a (128, TC/128) packed layout via DRAM-bounce
  repacks so it uses all lanes) -> TensorE broadcasts alpha/-delta rows
  across partitions with a ones-stationary matmul -> one VectorE pass
  p = x * bc_alpha -> TensorE identity-matmul injects p into PSUM
  accumulating onto bc(-delta) -> ScalarE evacuates with the per-channel
  affine fused: out = Wo_w * psum + Wo_b -> DMA out.
"""

import numpy as np

MOMENTUM = 0.05
EPS = 1e-06

B, C, T = 8, 512, 8192
NCT = C // 128          # 4 partition tiles of channels
TC = 1024               # time chunk
SUB = 512               # matmul / psum subchunk
NSC = TC // SUB
PJ = TC // 128          # packed free size per chunk

_PROG_CACHE = {}


def _build_program(t_total, reps=1):
    import concourse.bacc as bacc
    import concourse.bass as bass
    import concourse.tile as tile
    from concourse import mybir

    f32 = mybir.dt.float32
    f32r = mybir.dt.float32r
    AF = mybir.ActivationFunctionType
    OP = mybir.AluOpType

    nch = t_total // TC

    nc = bacc.Bacc(None, target_bir_lowering=False)

    x_d = nc.dram_tensor("x", [C, t_total], f32r, kind="ExternalInput")
    ag_d = nc.dram_tensor("ag_row", [2, t_total], f32, kind="ExternalInput")
    gt_d = nc.dram_tensor("gt_pack", [128, t_total // 128], f32,
                          kind="ExternalInput")
    w3a_d = nc.dram_tensor("w3a", [128, NCT, 3], f32r, kind="ExternalInput")
    w3b_d = nc.dram_tensor("w3b", [128, NCT, 3], f32r, kind="ExternalInput")
    wsc_d = nc.dram_tensor("wscale", [128, NCT], f32, kind="ExternalInput")
    wbi_d = nc.dram_tensor("wbias", [128, NCT], f32, kind="ExternalInput")
    out_d = nc.dram_tensor("out", [C, t_total], f32, kind="ExternalOutput")

    x_v = x_d.rearrange("(ct p) t -> p ct t", p=128)
    out_v = out_d.rearrange("(ct p) t -> p ct t", p=128)

    from contextlib import ExitStack

    with tile.TileContext(nc) as tc, ExitStack() as ctx, \
            nc.allow_low_precision(reason="float32r == fp32 bits in SBUF; "
                                   "PE rounds on load"):
        const = ctx.enter_context(tc.tile_pool(name="const", bufs=1))
        xp = ctx.enter_context(tc.tile_pool(name="xp", bufs=4))
        xsqp = ctx.enter_context(tc.tile_pool(name="xsqp", bufs=1))
        pp = ctx.enter_context(tc.tile_pool(name="pp", bufs=2))
        osb = ctx.enter_context(tc.tile_pool(name="osb", bufs=2))
        rows = ctx.enter_context(tc.tile_pool(name="rows", bufs=3))
        rows4 = ctx.enter_context(tc.tile_pool(name="rows4", bufs=3))
        agp = ctx.enter_context(tc.tile_pool(name="agp", bufs=2))
        pk = ctx.enter_context(tc.tile_pool(name="pk", bufs=6))
        ps_red = ctx.enter_context(
            tc.tile_pool(name="ps_red", bufs=1, space="PSUM"))
        ps_bca = ctx.enter_context(
            tc.tile_pool(name="ps_bca", bufs=1, space="PSUM"))
        ps_out = ctx.enter_context(
            tc.tile_pool(name="ps_out", bufs=1, space="PSUM"))

        # ---- constants ----
        ones_t = const.tile([128, 128], f32)
        nc.vector.memset(ones_t, 1.0)
        ident = const.tile([128, 128], f32r)
        # iota value = f - p; keep where ==0 -> identity matrix
        nc.gpsimd.affine_select(
            out=ident, in_=ones_t, pattern=[[1, 128]],
            compare_op=OP.is_equal, fill=0.0, base=0, channel_multiplier=-1,
        )
        ones_col = const.tile([1, 128], f32r)
        nc.vector.tensor_copy(out=ones_col, in_=ones_t[0:1, :])
        eps_t = const.tile([128, 1], f32)
        nc.vector.memset(eps_t, EPS)

        w3a = const.tile([128, NCT, 3], f32r)
        nc.sync.dma_start(out=w3a, in_=w3a_d[:])
        w3b = const.tile([128, NCT, 3], f32r)
        nc.sync.dma_start(out=w3b, in_=w3b_d[:])
        wsc = const.tile([128, NCT], f32)
        nc.sync.dma_start(out=wsc, in_=wsc_d[:])
        wbi = const.tile([128, NCT], f32)
        nc.sync.dma_start(out=wbi, in_=wbi_d[:])
        gt = const.tile([128, t_total // 128], f32)
        nc.sync.dma_start(out=gt, in_=gt_d[:])

        def pack(eng, row_ap, tag):
            """(1, TC) SBUF row -> (128, PJ) packed; direct partition-
            crossing DMA (APs iterate in matching flat order)."""
            p = pk.tile([128, PJ], f32, tag=tag)
            eng.dma_start(out=p, in_=row_ap)
            return p

        def unpack(eng, pk_ap, tag, dt=f32):
            """(128, PJ) packed -> (1, TC) SBUF row; direct DMA."""
            r = rows.tile([1, TC], dt, tag=tag)
            eng.dma_start(out=r, in_=pk_ap)
            return r

        rep_cm = (tc.For_i(0, reps, 1, staggered_reset=True)
                  if reps > 1 else None)
        if rep_cm is not None:
            rep_cm.__enter__()
        prev_mean = None
        prev_var = None
        st = {}

        def stage_a(k):
            """input + reductions + scans for chunk k."""
            nonlocal prev_mean, prev_var
            t0 = k * TC
            c = {}

            c["x_ch"] = x_ch = xp.tile([128, NCT, TC], f32r, name="x_ch")
            nc.sync.dma_start(out=x_ch, in_=x_v[:, :, t0:t0 + TC])
            ag_ch = agp.tile([1, 2, TC], f32, tag="ag", name="ag_ch")
            nc.sync.dma_start(out=ag_ch, in_=ag_d[:, t0:t0 + TC])
            a_ch = ag_ch[:, 0, :]
            gt_row = ag_ch[:, 1, :]

            red_ps = ps_red.tile([3, TC], f32, name="red_ps")
            for sx in range(NSC):
                sl = slice(sx * SUB, (sx + 1) * SUB)
                xsq = xsqp.tile([128, NCT, SUB], f32r, name="xsq")
                nc.scalar.activation(out=xsq, in_=x_ch[:, :, sl],
                                     func=AF.Square)
                for ct in range(NCT):
                    nc.tensor.matmul(
                        red_ps[:, sl], w3a[:, ct, :], x_ch[:, ct, sl],
                        start=(ct == 0), stop=False,
                    )
                for ct in range(NCT):
                    nc.tensor.matmul(
                        red_ps[:, sl], w3b[:, ct, :], xsq[:, ct, :],
                        start=False, stop=(ct == NCT - 1),
                    )
            red_rows = rows.tile([3, TC], f32, tag="red", name="red_rows")
            nc.scalar.activation(out=red_rows, in_=red_ps, func=AF.Copy)

            ub_p = pack(nc.scalar, red_rows[1:2, :], "ub")
            sb_p = pack(nc.scalar, red_rows[2:3, :], "sb")
            gt_k = gt[:, k * PJ:(k + 1) * PJ]

            b1_row = rows.tile([1, TC], f32, tag="b1r", name="b1_row")
            nc.vector.tensor_tensor(out=b1_row, in0=gt_row,
                                    in1=red_rows[0:1, :], op=OP.mult)
            mean_row = rows4.tile([1, TC], f32, tag="mean", name="mean_row")
            nc.vector.tensor_tensor_scan(
                out=mean_row, data0=a_ch, data1=b1_row,
                initial=(0.0 if k == 0 else prev_mean[:, TC - 1:TC]),
                op0=OP.mult, op1=OP.add,
            )
            c["mean_row"] = mean_row
            mean_p = pack(nc.gpsimd, mean_row, "mean")

            t1_p = pk.tile([128, PJ], f32, tag="t1", name="t1_p")
            nc.vector.scalar_tensor_tensor(
                out=t1_p, in0=ub_p, scalar=2.0, in1=mean_p,
                op0=OP.mult, op1=OP.subtract,
            )
            t2_p = pk.tile([128, PJ], f32, tag="t2", name="t2_p")
            nc.vector.tensor_tensor(out=t2_p, in0=t1_p, in1=mean_p,
                                    op=OP.mult)
            v_p = pk.tile([128, PJ], f32, tag="v", name="v_p")
            nc.vector.tensor_tensor(out=v_p, in0=sb_p, in1=t2_p,
                                    op=OP.subtract)
            b2_p = pk.tile([128, PJ], f32, tag="b2", name="b2_p")
            nc.vector.tensor_tensor(out=b2_p, in0=gt_k, in1=v_p, op=OP.mult)
            b2_row = unpack(nc.gpsimd, b2_p, "b2r")
            var_row = rows4.tile([1, TC], f32, tag="var", name="var_row")
            nc.vector.tensor_tensor_scan(
                out=var_row, data0=a_ch, data1=b2_row,
                initial=(0.0 if k == 0 else prev_var[:, TC - 1:TC]),
                op0=OP.mult, op1=OP.add,
            )
            c["var_p"] = pack(nc.gpsimd, var_row, "var")
            prev_mean, prev_var = mean_row, var_row
            return c

        def stage_b(k, c):
            """alpha/delta + broadcast + normalize + store for chunk k."""
            t0 = k * TC
            x_ch = c["x_ch"]

            sq_p = pk.tile([128, PJ], f32, tag="sq", name="sq_p")
            nc.scalar.activation(out=sq_p, in_=c["var_p"], func=AF.Sqrt,
                                 bias=eps_t)
            al_p = pk.tile([128, PJ], f32r, tag="al", name="al_p")
            nc.vector.reciprocal(out=al_p, in_=sq_p)
            al_row = unpack(nc.gpsimd, al_p, "alr", f32r)
            nd_row = rows.tile([1, TC], f32r, tag="ndr", name="nd_row")
            nc.vector.scalar_tensor_tensor(
                out=nd_row, in0=al_row, scalar=-1.0, in1=c["mean_row"],
                op0=OP.mult, op1=OP.mult,
            )

            bca = ps_bca.tile([128, TC], f32, name="bca")
            for sx in range(NSC):
                sl = slice(sx * SUB, (sx + 1) * SUB)
                nc.tensor.matmul(bca[:, sl], ones_col, al_row[:, sl],
                                 start=True, stop=True)

            for sx in range(NSC):
                sl = slice(sx * SUB, (sx + 1) * SUB)
                p_s = pp.tile([128, NCT, SUB], f32r, name="p_s")
                bca_sl = bca[:, sl]
                bca_bc = bass.AP(
                    tensor=bca_sl.tensor, offset=bca_sl.offset,
                    ap=[bca_sl.ap[0], [0, NCT], bca_sl.ap[1]],
                )
                nc.vector.scalar_tensor_tensor(
                    out=p_s, in0=x_ch[:, :, sl], scalar=1.0,
                    in1=bca_bc, op0=OP.mult, op1=OP.mult,
                )
                out_sb = osb.tile([128, NCT, SUB], f32, name="out_sb")
                ops_l = [ps_out.tile([128, SUB], f32, tag=f"o{ct}",
                                     name=f"ops{ct}")
                         for ct in range(NCT)]
                for ct in range(NCT):
                    nc.tensor.matmul(ops_l[ct], ones_col, nd_row[:, sl],
                                     start=True, stop=False)
                for ct in range(NCT):
                    nc.tensor.matmul(ops_l[ct], ident, p_s[:, ct, :],
                                     start=False, stop=True)
                for ct in range(NCT):
                    nc.scalar.activation(
                        out=out_sb[:, ct, :], in_=ops_l[ct],
                        func=AF.Identity,
                        scale=wsc[:, ct:ct + 1], bias=wbi[:, ct:ct + 1],
                    )
                nc.scalar.dma_start(
                    out=out_v[:, :, t0 + sx * SUB:t0 + (sx + 1) * SUB],
                    in_=out_sb)

        pri_marks = []
        for k in range(nch + 1):
            pri_marks.append(tc.cur_priority)
            if k < nch:
                st[k] = stage_a(k)
            if k >= 1:
                # sort stage B one full iteration later than emitted so
                # in-order engine queues never block early chunk k+1 work
                # behind chunk k-1 tail waits
                span = (pri_marks[-1] - pri_marks[-2]) if k >= 2 else 0
                with tc.high_priority(offset=-span):
                    stage_b(k - 1, st.pop(k - 1))
        if rep_cm is not None:
            rep_cm.__exit__(None, None, None)

    nc.compile()
    return nc


def _get_program(t_total=T, reps=1):
    key = (t_total, reps)
    if key not in _PROG_CACHE:
        _PROG_CACHE[key] = _build_program(t_total, reps)
    return _PROG_CACHE[key]


def _host_prep(x, g, Wa_w, Wb_w, Wo_w, Wo_b, t_total):
    """Build per-core input maps (host does only O(B*T + C) work)."""
    x = np.asarray(x, np.float32)
    g = np.asarray(g, np.float32)
    wa = np.asarray(Wa_w, np.float32).reshape(C)
    wb = np.asarray(Wb_w, np.float32).reshape(C)
    wo_w = np.asarray(Wo_w, np.float32).reshape(C)
    wo_b = np.asarray(Wo_b, np.float32).reshape(C)

    def softmax(v):
        e = np.exp(v - v.max())
        return (e / e.sum()).astype(np.float32)

    wa, wb = softmax(wa), softmax(wb)
    zero = np.zeros_like(wa)
    w3a = np.stack([wa, wb, zero], 1).reshape(NCT, 128, 3).transpose(1, 0, 2)
    w3b = np.stack([zero, zero, wb], 1).reshape(NCT, 128, 3).transpose(1, 0, 2)
    wsc = wo_w.reshape(NCT, 128).T
    wbi = wo_b.reshape(NCT, 128).T

    shared = {
        "w3a": np.ascontiguousarray(w3a),
        "w3b": np.ascontiguousarray(w3b),
        "wscale": np.ascontiguousarray(wsc),
        "wbias": np.ascontiguousarray(wbi),
    }
    nch = t_total // TC
    in_maps = []
    for b in range(x.shape[0]):
        gt = (g[b, 0, :] * MOMENTUM).astype(np.float32)
        ag_row = np.stack([1.0 - gt, gt], 0)
        gt_pack = np.ascontiguousarray(
            gt.reshape(nch, 128, PJ).transpose(1, 0, 2).reshape(128, -1))
        in_maps.append({
            "x": np.ascontiguousarray(x[b]),
            "ag_row": np.ascontiguousarray(ag_row),
            "gt_pack": gt_pack,
            **shared,
        })
    return in_maps


LAST_RESULTS = None


def kernel(x, g, Wa_w, Wb_w, Wo_w, Wo_b):
    global LAST_RESULTS
    from concourse.bass_utils import run_bass_kernel_spmd

    t_total = x.shape[2]
    nc = _get_program(t_total)
    in_maps = _host_prep(x, g, Wa_w, Wb_w, Wo_w, Wo_b, t_total)
    n = len(in_maps)
    res = run_bass_kernel_spmd(nc, in_maps, list(range(n)))
    LAST_RESULTS = res
    return np.stack([res.results[i]["out"] for i in range(n)], 0)


# revision 39
# speedup vs baseline: 3.2539x; 1.0641x over previous
"""GatedTSNorm Trainium2 kernel.

Math: the gated EMA y[t] = (1-g~[t])*y[t-1] + g~[t]*v[t] is linear with a
gate shared across channels, so channel-weighted sums commute with it:
    mean[b,t] = EMA(sum_c wa_c x[b,c,t])
    var[b,t]  = EMA(sum_c wb_c (x-mean)^2) = EMA(sb - mean*(2*ub - mean))
with ua/ub/sb the wa/wb-weighted reductions of x and x^2 over C
(softmax weights sum to 1). Output:
    out[c,t] = Wo_w[c] * (x[c,t] - mean[t]) * rsqrt(var[t]+eps) + Wo_b[c]
             = Wo_w[c] * (alpha[t]*x[c,t] + (-alpha[t]*mean[t])) + Wo_b[c]

Sharding: batch B=8 -> one batch per NeuronCore, zero communication.

Per-core pipeline over T chunks (c-major layout, C=512 on 4 partition
tiles):
  DMA in -> ScalarE x^2 -> TensorE reductions (one fused PSUM group,
  stationary [wa|wb|0] on x and [0|0|wb] on x^2, float32r moving) ->
  VectorE tensor_tensor_scan for the two EMAs (tiny per-(b,t) rows;
  elementwise prep runs in a (128, TC/128) packed layout via DRAM-bounce
  repacks so it uses all lanes) -> TensorE broadcasts alpha/-delta rows
  across partitions with a ones-stationary matmul -> one VectorE pass
  p = x * bc_alpha -> TensorE identity-matmul injects p into PSUM
  accumulating onto bc(-delta) -> ScalarE evacuates with the per-channel
  affine fused: out = Wo_w * psum + Wo_b -> DMA out.
"""

import numpy as np

MOMENTUM = 0.05
EPS = 1e-06

B, C, T = 8, 512, 8192
NCT = C // 128          # 4 partition tiles of channels
TC = 1024               # time chunk
SUB = 512               # matmul / psum subchunk
NSC = TC // SUB
PJ = TC // 128          # packed free size per chunk

_PROG_CACHE = {}


def _build_program(t_total, reps=1):
    import concourse.bacc as bacc
    import concourse.bass as bass
    import concourse.tile as tile
    from concourse import mybir

    f32 = mybir.dt.float32
    f32r = mybir.dt.float32r
    AF = mybir.ActivationFunctionType
    OP = mybir.AluOpType

    nch = t_total // TC

    nc = bacc.Bacc(None, target_bir_lowering=False)

    x_d = nc.dram_tensor("x", [C, t_total], f32r, kind="ExternalInput")
    ag_d = nc.dram_tensor("ag_row", [2, t_total], f32, kind="ExternalInput")
    gt_d = nc.dram_tensor("gt_pack", [128, t_total // 128], f32,
                          kind="ExternalInput")
    w3a_d = nc.dram_tensor("w3a", [128, NCT, 3], f32r, kind="ExternalInput")
    w3b_d = nc.dram_tensor("w3b", [128, NCT, 3], f32r, kind="ExternalInput")
    wsc_d = nc.dram_tensor("wscale", [128, NCT], f32, kind="ExternalInput")
    wbi_d = nc.dram_tensor("wbias", [128, NCT], f32, kind="ExternalInput")
    out_d = nc.dram_tensor("out", [C, t_total], f32, kind="ExternalOutput")

    x_v = x_d.rearrange("(ct p) t -> p ct t", p=128)
    out_v = out_d.rearrange("(ct p) t -> p ct t", p=128)

    from contextlib import ExitStack

    with tile.TileContext(nc) as tc, ExitStack() as ctx, \
            nc.allow_low_precision(reason="float32r == fp32 bits in SBUF; "
                                   "PE rounds on load"):
        const = ctx.enter_context(tc.tile_pool(name="const", bufs=1))
        xp = ctx.enter_context(tc.tile_pool(name="xp", bufs=4))
        xsqp = ctx.enter_context(tc.tile_pool(name="xsqp", bufs=1))
        pp = ctx.enter_context(tc.tile_pool(name="pp", bufs=2))
        osb = ctx.enter_context(tc.tile_pool(name="osb", bufs=2))
        rows = ctx.enter_context(tc.tile_pool(name="rows", bufs=3))
        rows4 = ctx.enter_context(tc.tile_pool(name="rows4", bufs=3))
        agp = ctx.enter_context(tc.tile_pool(name="agp", bufs=2))
        pk = ctx.enter_context(tc.tile_pool(name="pk", bufs=6))
        ps_red = ctx.enter_context(
            tc.tile_pool(name="ps_red", bufs=2, space="PSUM"))
        ps_bca = ctx.enter_context(
            tc.tile_pool(name="ps_bca", bufs=1, space="PSUM"))
        ps_out = ctx.enter_context(
            tc.tile_pool(name="ps_out", bufs=1, space="PSUM"))

        # ---- constants ----
        ones_t = const.tile([128, 128], f32)
        nc.vector.memset(ones_t, 1.0)
        ident = const.tile([128, 128], f32r)
        # iota value = f - p; keep where ==0 -> identity matrix
        nc.gpsimd.affine_select(
            out=ident, in_=ones_t, pattern=[[1, 128]],
            compare_op=OP.is_equal, fill=0.0, base=0, channel_multiplier=-1,
        )
        ones_col = const.tile([1, 128], f32r)
        nc.vector.tensor_copy(out=ones_col, in_=ones_t[0:1, :])
        eps_t = const.tile([128, 1], f32)
        nc.vector.memset(eps_t, EPS)

        w3a = const.tile([128, NCT, 3], f32r)
        nc.sync.dma_start(out=w3a, in_=w3a_d[:])
        w3b = const.tile([128, NCT, 3], f32r)
        nc.sync.dma_start(out=w3b, in_=w3b_d[:])
        wsc = const.tile([128, NCT], f32)
        nc.sync.dma_start(out=wsc, in_=wsc_d[:])
        wbi = const.tile([128, NCT], f32)
        nc.sync.dma_start(out=wbi, in_=wbi_d[:])
        gt = const.tile([128, t_total // 128], f32)
        nc.sync.dma_start(out=gt, in_=gt_d[:])

        def pack(eng, row_ap, tag):
            """(1, TC) SBUF row -> (128, PJ) packed; direct partition-
            crossing DMA (APs iterate in matching flat order)."""
            p = pk.tile([128, PJ], f32, tag=tag)
            eng.dma_start(out=p, in_=row_ap)
            return p

        def unpack(eng, pk_ap, tag, dt=f32):
            """(128, PJ) packed -> (1, TC) SBUF row; direct DMA."""
            r = rows.tile([1, TC], dt, tag=tag)
            eng.dma_start(out=r, in_=pk_ap)
            return r

        rep_cm = (tc.For_i(0, reps, 1, staggered_reset=True)
                  if reps > 1 else None)
        if rep_cm is not None:
            rep_cm.__enter__()
        prev_mean = None
        prev_var = None
        st = {}

        def stage_a(k):
            """input + reductions + scans for chunk k."""
            nonlocal prev_mean, prev_var
            t0 = k * TC
            c = {}

            c["x_ch"] = x_ch = xp.tile([128, NCT, TC], f32r, name="x_ch")
            nc.sync.dma_start(out=x_ch, in_=x_v[:, :, t0:t0 + TC])
            ag_ch = agp.tile([1, 2, TC], f32, tag="ag", name="ag_ch")
            nc.sync.dma_start(out=ag_ch, in_=ag_d[:, t0:t0 + TC])
            a_ch = ag_ch[:, 0, :]
            gt_row = ag_ch[:, 1, :]

            red_rows = rows.tile([3, TC], f32, tag="red", name="red_rows")
            for sx in range(NSC):
                sl = slice(sx * SUB, (sx + 1) * SUB)
                xsq = xsqp.tile([128, NCT, SUB], f32r, name="xsq")
                nc.scalar.activation(out=xsq, in_=x_ch[:, :, sl],
                                     func=AF.Square)
                red_ps = ps_red.tile([3, SUB], f32, name="red_ps")
                for ct in range(NCT):
                    nc.tensor.matmul(
                        red_ps, w3a[:, ct, :], x_ch[:, ct, sl],
                        start=(ct == 0), stop=False,
                    )
                for ct in range(NCT):
                    nc.tensor.matmul(
                        red_ps, w3b[:, ct, :], xsq[:, ct, :],
                        start=False, stop=(ct == NCT - 1),
                    )
                nc.scalar.activation(out=red_rows[:, sl], in_=red_ps,
                                     func=AF.Copy)

            ub_p = pack(nc.scalar, red_rows[1:2, :], "ub")
            sb_p = pack(nc.scalar, red_rows[2:3, :], "sb")
            gt_k = gt[:, k * PJ:(k + 1) * PJ]

            b1_row = rows.tile([1, TC], f32, tag="b1r", name="b1_row")
            nc.vector.tensor_tensor(out=b1_row, in0=gt_row,
                                    in1=red_rows[0:1, :], op=OP.mult)
            mean_row = rows4.tile([1, TC], f32, tag="mean", name="mean_row")
            nc.vector.tensor_tensor_scan(
                out=mean_row, data0=a_ch, data1=b1_row,
                initial=(0.0 if k == 0 else prev_mean[:, TC - 1:TC]),
                op0=OP.mult, op1=OP.add,
            )
            c["mean_row"] = mean_row
            mean_p = pack(nc.gpsimd, mean_row, "mean")

            t1_p = pk.tile([128, PJ], f32, tag="t1", name="t1_p")
            nc.vector.scalar_tensor_tensor(
                out=t1_p, in0=ub_p, scalar=2.0, in1=mean_p,
                op0=OP.mult, op1=OP.subtract,
            )
            t2_p = pk.tile([128, PJ], f32, tag="t2", name="t2_p")
            nc.vector.tensor_tensor(out=t2_p, in0=t1_p, in1=mean_p,
                                    op=OP.mult)
            v_p = pk.tile([128, PJ], f32, tag="v", name="v_p")
            nc.vector.tensor_tensor(out=v_p, in0=sb_p, in1=t2_p,
                                    op=OP.subtract)
            b2_p = pk.tile([128, PJ], f32, tag="b2", name="b2_p")
            nc.vector.tensor_tensor(out=b2_p, in0=gt_k, in1=v_p, op=OP.mult)
            b2_row = unpack(nc.gpsimd, b2_p, "b2r")
            var_row = rows4.tile([1, TC], f32, tag="var", name="var_row")
            nc.vector.tensor_tensor_scan(
                out=var_row, data0=a_ch, data1=b2_row,
                initial=(0.0 if k == 0 else prev_var[:, TC - 1:TC]),
                op0=OP.mult, op1=OP.add,
            )
            c["var_p"] = pack(nc.gpsimd, var_row, "var")
            prev_mean, prev_var = mean_row, var_row
            return c

        def stage_b(k, c):
            """alpha/delta + broadcast + normalize + store for chunk k."""
            t0 = k * TC
            x_ch = c["x_ch"]

            sq_p = pk.tile([128, PJ], f32, tag="sq", name="sq_p")
            nc.scalar.activation(out=sq_p, in_=c["var_p"], func=AF.Sqrt,
                                 bias=eps_t)
            al_p = pk.tile([128, PJ], f32r, tag="al", name="al_p")
            nc.vector.reciprocal(out=al_p, in_=sq_p)
            al_row = unpack(nc.gpsimd, al_p, "alr", f32r)
            nd_row = rows.tile([1, TC], f32r, tag="ndr", name="nd_row")
            nc.vector.scalar_tensor_tensor(
                out=nd_row, in0=al_row, scalar=-1.0, in1=c["mean_row"],
                op0=OP.mult, op1=OP.mult,
            )

            bca = ps_bca.tile([128, TC], f32, name="bca")
            for sx in range(NSC):
                sl = slice(sx * SUB, (sx + 1) * SUB)
                nc.tensor.matmul(bca[:, sl], ones_col, al_row[:, sl],
                                 start=True, stop=True)

            for sx in range(NSC):
                sl = slice(sx * SUB, (sx + 1) * SUB)
                p_s = pp.tile([128, NCT, SUB], f32r, name="p_s")
                bca_sl = bca[:, sl]
                bca_bc = bass.AP(
                    tensor=bca_sl.tensor, offset=bca_sl.offset,
                    ap=[bca_sl.ap[0], [0, NCT], bca_sl.ap[1]],
                )
                nc.vector.scalar_tensor_tensor(
                    out=p_s, in0=x_ch[:, :, sl], scalar=1.0,
                    in1=bca_bc, op0=OP.mult, op1=OP.mult,
                )
                out_sb = osb.tile([128, NCT, SUB], f32, name="out_sb")
                ops_l = [ps_out.tile([128, SUB], f32, tag=f"o{ct}",
                                     name=f"ops{ct}")
                         for ct in range(NCT)]
                for ct in range(NCT):
                    nc.tensor.matmul(ops_l[ct], ones_col, nd_row[:, sl],
                                     start=True, stop=False)
                for ct in range(NCT):
                    nc.tensor.matmul(ops_l[ct], ident, p_s[:, ct, :],
                                     start=False, stop=True)
                for ct in range(NCT):
                    nc.scalar.activation(
                        out=out_sb[:, ct, :], in_=ops_l[ct],
                        func=AF.Identity,
                        scale=wsc[:, ct:ct + 1], bias=wbi[:, ct:ct + 1],
                    )
                nc.scalar.dma_start(
                    out=out_v[:, :, t0 + sx * SUB:t0 + (sx + 1) * SUB],
                    in_=out_sb)

        pri_marks = []
        for k in range(nch + 1):
            pri_marks.append(tc.cur_priority)
            if k < nch:
                st[k] = stage_a(k)
            if k >= 1:
                # sort stage B one full iteration later than emitted so
                # in-order engine queues never block early chunk k+1 work
                # behind chunk k-1 tail waits
                span = (pri_marks[-1] - pri_marks[-2]) if k >= 2 else 0
                with tc.high_priority(offset=-span):
                    stage_b(k - 1, st.pop(k - 1))
        if rep_cm is not None:
            rep_cm.__exit__(None, None, None)

    nc.compile()
    return nc


def _get_program(t_total=T, reps=1):
    key = (t_total, reps)
    if key not in _PROG_CACHE:
        _PROG_CACHE[key] = _build_program(t_total, reps)
    return _PROG_CACHE[key]


def _host_prep(x, g, Wa_w, Wb_w, Wo_w, Wo_b, t_total):
    """Build per-core input maps (host does only O(B*T + C) work)."""
    x = np.asarray(x, np.float32)
    g = np.asarray(g, np.float32)
    wa = np.asarray(Wa_w, np.float32).reshape(C)
    wb = np.asarray(Wb_w, np.float32).reshape(C)
    wo_w = np.asarray(Wo_w, np.float32).reshape(C)
    wo_b = np.asarray(Wo_b, np.float32).reshape(C)

    def softmax(v):
        e = np.exp(v - v.max())
        return (e / e.sum()).astype(np.float32)

    wa, wb = softmax(wa), softmax(wb)
    zero = np.zeros_like(wa)
    w3a = np.stack([wa, wb, zero], 1).reshape(NCT, 128, 3).transpose(1, 0, 2)
    w3b = np.stack([zero, zero, wb], 1).reshape(NCT, 128, 3).transpose(1, 0, 2)
    wsc = wo_w.reshape(NCT, 128).T
    wbi = wo_b.reshape(NCT, 128).T

    shared = {
        "w3a": np.ascontiguousarray(w3a),
        "w3b": np.ascontiguousarray(w3b),
        "wscale": np.ascontiguousarray(wsc),
        "wbias": np.ascontiguousarray(wbi),
    }
    nch = t_total // TC
    in_maps = []
    for b in range(x.shape[0]):
        gt = (g[b, 0, :] * MOMENTUM).astype(np.float32)
        ag_row = np.stack([1.0 - gt, gt], 0)
        gt_pack = np.ascontiguousarray(
            gt.reshape(nch, 128, PJ).transpose(1, 0, 2).reshape(128, -1))
        in_maps.append({
            "x": np.ascontiguousarray(x[b]),
            "ag_row": np.ascontiguousarray(ag_row),
            "gt_pack": gt_pack,
            **shared,
        })
    return in_maps


LAST_RESULTS = None


def kernel(x, g, Wa_w, Wb_w, Wo_w, Wo_b):
    global LAST_RESULTS
    from concourse.bass_utils import run_bass_kernel_spmd

    t_total = x.shape[2]
    nc = _get_program(t_total)
    in_maps = _host_prep(x, g, Wa_w, Wb_w, Wo_w, Wo_b, t_total)
    n = len(in_maps)
    res = run_bass_kernel_spmd(nc, in_maps, list(range(n)))
    LAST_RESULTS = res
    return np.stack([res.results[i]["out"] for i in range(n)], 0)
